# revision 51
# baseline (speedup 1.0000x reference)
import hashlib
import os
import numpy as np
import ml_dtypes

# nn_MultiHeadAttention: B=4, S=2048, D=1024, HEADS=16, DIM_HEAD=64.
# Sharding: batch (4) x head-group (2) across 8 cores. Each core computes
# attention for one batch and 8 heads, plus its partial of the output
# projection (row-parallel W0); the two head-group partials per batch are
# summed on the host.
#
# Scheduling notes (from trace analysis):
# - matmuls into the SAME psum bank serialize at ~373ns; alternating banks
#   pipeline at ~216ns -> all accumulation chains are 2-way interleaved.
# - the attention jg loop is ACT-bound (2x1112ns exp vs ~1500ns of matmul);
#   projection work is injected one matmul at a time between AV matmuls.
B, S, D = 4, 2048, 1024
HEADS, DH = 16, 64
HPC = 8               # heads per core
E = HPC * DH          # 512 local projection channels
SCALE = DH ** -0.5
P = 128
MT = D // P           # 8 contraction tiles
NPAIR = HPC // 2      # 4 head pairs (= e-chunks of 128)
NTB = S // 512        # 4 query blocks of 512
NJT = S // P          # 16 key tiles of 128
VPW = 65 + 128        # vp columns per pair: h0 [V|1], h1 [1|0*63|V]

_CACHE = {}


def _build():
    if "nc" in _CACHE:
        return _CACHE["nc"]
    import concourse.bacc as bacc
    import concourse.mybir as mybir
    from concourse.tile import TileContext

    f32 = mybir.dt.float32
    bf16 = mybir.dt.bfloat16
    EXP = mybir.ActivationFunctionType.Exp
    MULT = mybir.AluOpType.mult

    nc = bacc.Bacc("TRN2", target_bir_lowering=False, debug=False, num_devices=8)
    qT_d = nc.dram_tensor("qT", [D, S], bf16, kind="ExternalInput")
    kvT_d = nc.dram_tensor("kvT", [D, S], bf16, kind="ExternalInput")
    wq_d = nc.dram_tensor("wqT", [D, E], bf16, kind="ExternalInput")
    wk_d = nc.dram_tensor("wkT", [D, E], bf16, kind="ExternalInput")
    wv_d = nc.dram_tensor("wvT", [D, E], bf16, kind="ExternalInput")
    w0_d = nc.dram_tensor("w0a", [E, D], bf16, kind="ExternalInput")
    out_d = nc.dram_tensor("poutT", [D, S], bf16, kind="ExternalOutput")
    # The NEFF cache keys on the HLO signature but NOT the embedded BIR, so
    # two kernel versions with identical I/O signatures collide and a stale
    # NEFF gets silently reused. Encode a hash of this source file into a
    # dummy output's shape so every kernel edit changes the signature.
    try:
        with open(__file__, "rb") as f:
            _h = int(hashlib.sha256(f.read()).hexdigest()[:8], 16)
    except OSError:
        _h = 0
    SIG_MAGIC = float(1000 + _h % 509)
    _CACHE["sig_magic"] = SIG_MAGIC
    sig_d = nc.dram_tensor("sig", [1, 2 + _h % 509], f32, kind="ExternalOutput")
    DEBUG_DUMP = bool(int(os.environ.get("KERNEL_DEBUG_DUMP", "0")))
    dbg_d = dbg2_d = None
    if DEBUG_DUMP:
        # rows 0:65 pn0, 65:193 pn1, 193 rrowf(1024)
        dbg_d = nc.dram_tensor("dbg", [323, 1024], f32, kind="ExternalOutput")
        dbg2_d = nc.dram_tensor("dbg2", [P, 512], bf16, kind="ExternalOutput")

    with TileContext(nc) as tc:
        with (
            tc.tile_pool(name="pers", bufs=1) as pers,
            tc.tile_pool(name="psS", bufs=1, space="PSUM") as psS,
            tc.tile_pool(name="psO", bufs=1, space="PSUM") as psO,
        ):
            # ---- persistent SBUF tiles (live across phases) ----
            sig_t = pers.tile([1, 8], f32, tag="sig", name="sig")
            nc.gpsimd.memset(sig_t[:, :], SIG_MAGIC)
            nc.sync.dma_start(out=sig_d[0:1, 0:2], in_=sig_t[0:1, 0:2])
            w0a = [pers.tile([P, D], bf16, tag=f"w0{p}", name=f"w0{p}") for p in range(NPAIR)]
            qpt = [pers.tile([P, S], bf16, tag=f"qp{p}", name=f"qp{p}") for p in range(NPAIR)]
            kpt = [pers.tile([P, S], bf16, tag=f"kp{p}", name=f"kp{p}") for p in range(NPAIR)]
            vp = [pers.tile([P, NPAIR * VPW], bf16, tag=f"vp{t}", name=f"vp{t}") for t in range(NJT)]
            onorm = [pers.tile([P, S], bf16, tag=f"on{p}", name=f"on{p}") for p in range(NPAIR)]
            for p in range(NPAIR):
                nc.sync.dma_start(out=w0a[p][:, :], in_=w0_d[p * P:(p + 1) * P, :])

            # psum allocation for projection chains. Outside attention all
            # tags are free (5-entry rotation); during attention a chain must
            # use an explicit po tag (o0/o1) — its other buffer — since a
            # long-lived chain on a sAB buffer would stall the scores/exp
            # pipeline. At most one chain alloc per tag per ibl window, or the
            # alloc rotation hands out the live po buffer and deadlocks.
            ptags_free = [(psO, "o0"), (psS, "sAB"), (psO, "o1"),
                          (psS, "sAB"), (psS, "sAB")]
            pi_state = [0]

            def proj_psum(sel="free"):
                if sel == "free":
                    pool, tag = ptags_free[pi_state[0] % len(ptags_free)]
                    pi_state[0] += 1
                else:
                    pool, tag = psO, sel
                return pool.tile([P, 512], f32, tag=tag, name=f"proj_{tag}", bufs=2)

            with tc.tile_pool(name="phA", bufs=1) as pha:
                qTt = [pha.tile([P, S], bf16, tag=f"qT{i}", name=f"qT{i}") for i in range(MT)]
                kvTt = [pha.tile([P, S], bf16, tag=f"kvT{i}", name=f"kvT{i}") for i in range(MT)]
                wqt = [pha.tile([P, E], bf16, tag=f"wq{i}", name=f"wq{i}") for i in range(MT)]
                wkt = [pha.tile([P, E], bf16, tag=f"wk{i}", name=f"wk{i}") for i in range(MT)]
                wvt = [pha.tile([P, E], bf16, tag=f"wv{i}", name=f"wv{i}") for i in range(MT)]

                # DMA in consumption order: wv + kvT column-block 0 feed the
                # first vp chains; wk then the kp chains; then the rest.
                # alternate input loads across both HWDGE queues (sync +
                # scalar engines) — the pre-phase is input-bandwidth-bound
                # and the scalar engine is idle until the first exp
                def in_dma(i, out, src):
                    eng = nc.sync if i % 2 == 0 else nc.scalar
                    eng.dma_start(out=out, in_=src)

                for i in range(MT):
                    in_dma(i, wvt[i][:, :], wv_d[i * P:(i + 1) * P, :])
                for i in range(MT):
                    in_dma(i, kvTt[i][:, 0:512], kvT_d[i * P:(i + 1) * P, 0:512])
                for i in range(MT):
                    in_dma(i, wkt[i][:, :], wk_d[i * P:(i + 1) * P, :])
                for i in range(MT):
                    in_dma(i, kvTt[i][:, 512:1024], kvT_d[i * P:(i + 1) * P, 512:1024])
                for i in range(MT):
                    in_dma(i, kvTt[i][:, 1024:2048], kvT_d[i * P:(i + 1) * P, 1024:2048])
                for i in range(MT):
                    in_dma(i, wqt[i][:, :], wq_d[i * P:(i + 1) * P, :])
                    in_dma(i + 1, qTt[i][:, 0:512], qT_d[i * P:(i + 1) * P, 0:512])
                # late loads overlap attention: sync queue only, so the
                # scalar engine's DMA slices don't steal exp time
                for i in range(MT):
                    nc.sync.dma_start(out=qTt[i][:, 512:S], in_=qT_d[i * P:(i + 1) * P, 512:S])

                # ---------- chain generators: yield one step per call ----------
                # A chain = psum alloc + 8 (or 4) accumulating matmuls + finish.
                # Steps from two live chains are interleaved so consecutive
                # matmuls hit different psum banks.

                def vp_chain(t, rot="free", ctx=None):
                    st = ctx if ctx is not None else {"ps": None}

                    def step(i):
                        if i == 0:
                            nc.gpsimd.memset(vp[t][:, :], 0.0)
                            v3 = vp[t].rearrange("x (g c) -> x g c", c=VPW)
                            nc.gpsimd.memset(v3[:, :, 64:66], 1.0)
                            if st["ps"] is None:
                                st["ps"] = proj_psum(rot)
                        nc.tensor.matmul(
                            st["ps"][:, :],
                            lhsT=kvTt[i][:, t * P:(t + 1) * P],
                            rhs=wvt[i][:, :],
                            start=(i == 0), stop=(i == MT - 1),
                        )
                        if i == MT - 1:
                            v3 = vp[t].rearrange("x (g c) -> x g c", c=VPW)
                            p3 = st["ps"].rearrange("x (g c) -> x g c", c=P)
                            nc.vector.tensor_copy(out=v3[:, :, 0:64], in_=p3[:, :, 0:64])
                            nc.vector.tensor_copy(out=v3[:, :, 129:193], in_=p3[:, :, 64:128])
                    return [lambda i=i: step(i) for i in range(MT)]

                def proj_chain(dst, wt, xt, ec, tb, rot="free", ctx=None):
                    st = ctx if ctx is not None else {"ps": None}

                    def step(i):
                        if i == 0 and st["ps"] is None:
                            st["ps"] = proj_psum(rot)
                        nc.tensor.matmul(
                            st["ps"][:, :],
                            lhsT=wt[i][:, ec * P:(ec + 1) * P],
                            rhs=xt[i][:, tb * 512:(tb + 1) * 512],
                            start=(i == 0), stop=(i == MT - 1),
                        )
                        if i == MT - 1:
                            nc.vector.tensor_copy(
                                out=dst[ec][:, tb * 512:(tb + 1) * 512], in_=st["ps"][:, :])
                    return [lambda i=i: step(i) for i in range(MT)]

                def final_chain(dc, tb, rot="free", ctx=None):
                    # ctx: a {"ps": tile-or-None} holder shared by several
                    # sequential final chains; reusing one psum alloc avoids
                    # extra pool-rotation allocs (which can deadlock against a
                    # live po buffer mid-ibl).
                    st = ctx if ctx is not None else {"ps": None}

                    def step(i):
                        if i == 0 and st["ps"] is None:
                            st["ps"] = proj_psum(rot)
                        nc.tensor.matmul(
                            st["ps"][:, :],
                            lhsT=w0a[i][:, dc * P:(dc + 1) * P],
                            rhs=onorm[i][:, tb * 512:(tb + 1) * 512],
                            start=(i == 0), stop=(i == NPAIR - 1),
                        )
                        if i == NPAIR - 1:
                            ob = obp.tile([P, 512], bf16, tag="ob", name="ob")
                            nc.vector.tensor_copy(out=ob[:, :], in_=st["ps"][:, :])
                            nc.sync.dma_start(
                                out=out_d[dc * P:(dc + 1) * P, tb * 512:(tb + 1) * 512],
                                in_=ob[:, :])
                    return [lambda i=i: step(i) for i in range(NPAIR)]

                def run_pair(chA, chB, interleave=True):
                    if not interleave:
                        for s in chA:
                            s()
                        for s in chB:
                            s()
                        return
                    la, lb = len(chA), len(chB)
                    for i in range(max(la, lb)):
                        if i < la:
                            chA[i]()
                        if i < lb:
                            chB[i]()

                with (
                    tc.tile_pool(name="at", bufs=3) as atp,
                    tc.tile_pool(name="small", bufs=2) as small,
                    tc.tile_pool(name="ob", bufs=3) as obp,
                ):
                    # ---------- pre-phase: the minimum pair-0 ibl-0 needs ----------
                    # vp8..13 are deferred into ibl0's slack (each completes a
                    # jg before its AV consumer); vp14/15 stay here since the
                    # injection rate can't finish them by jg6.
                    ILV_PRE = True
                    for t in range(0, 8, 2):
                        run_pair(vp_chain(t), vp_chain(t + 1), interleave=ILV_PRE)
                    run_pair(proj_chain(kpt, wkt, kvTt, 0, 0),
                             proj_chain(kpt, wkt, kvTt, 0, 1), interleave=ILV_PRE)
                    run_pair(proj_chain(kpt, wkt, kvTt, 0, 2),
                             proj_chain(kpt, wkt, kvTt, 0, 3), interleave=ILV_PRE)
                    run_pair(vp_chain(14), vp_chain(15), interleave=ILV_PRE)
                    run_pair(proj_chain(qpt, wqt, qTt, 0, 0),
                             proj_chain(qpt, wqt, qTt, 0, 1), interleave=ILV_PRE)

                    # ---------- extras: per-(pair, ibl) step queues ----------
                    # During pair p's attention we inject projection work one
                    # matmul at a time between AV matmuls. Each ibl window has
                    # two lanes (psum tags o0/o1, the po buffers' second
                    # buffer); a lane's chains run sequentially, lanes are
                    # interleaved element-wise for psum-bank alternation.
                    def kp_ch(pn, tb, tag):
                        return proj_chain(kpt, wkt, kvTt, pn, tb, rot=tag)

                    def qp_ch(pn, tb, tag):
                        return proj_chain(qpt, wqt, qTt, pn, tb, rot=tag)

                    # flat queue per pair; chains run sequentially (one live
                    # at a time), alternating psum tags o0/o1 per chain. Pull
                    # counts per ibl are sized so each chain lands in the ibl
                    # window its outputs are needed after (and finals only
                    # start once their onorm block exists).
                    def interleave2(qA, qB):
                        q = []
                        for i in range(max(len(qA), len(qB))):
                            if i < len(qA):
                                q.append(qA[i])
                            if i < len(qB):
                                q.append(qB[i])
                        return q

                    extras = {}
                    rates = {}
                    # pair 0 ibl0: vp8..13 on two reused psum ctxs at 8/jg
                    ctxA, ctxB = {"ps": None}, {"ps": None}
                    qA, qB = [], []
                    for t in range(8, 14, 2):
                        qA.extend(vp_chain(t, rot="o0", ctx=ctxA))
                        qB.extend(vp_chain(t + 1, rot="o1", ctx=ctxB))
                    q0 = interleave2(qA, qB)
                    # ibl1: kp1 tb0+tb1 share one o0 ctx; qp0-tb2 on o1.
                    # ibl2 likewise with kp1 tb2+tb3 and qp0-tb3.
                    ctxK1, ctxK2 = {"ps": None}, {"ps": None}
                    q0 += interleave2(
                        [s for tb in (0, 1) for s in proj_chain(
                            kpt, wkt, kvTt, 1, tb, rot="o0", ctx=ctxK1)],
                        qp_ch(0, 2, "o1"))
                    ctxQ0 = {"ps": None}
                    q0 += interleave2(
                        [s for tb in (2, 3) for s in proj_chain(
                            kpt, wkt, kvTt, 1, tb, rot="o0", ctx=ctxK2)],
                        [s for args in ((0, 3), (1, 0)) for s in proj_chain(
                            qpt, wqt, qTt, args[0], args[1], rot="o1", ctx=ctxQ0)])
                    q0 += qp_ch(1, 1, "o0")
                    extras[0] = q0
                    rates[(0, 0)] = [8, 8, 8, 8, 8, 8, 0, 0]
                    rates[(0, 1)] = 3
                    rates[(0, 2)] = 4
                    rates[(0, 3)] = 1
                    for pp in (1, 2):
                        ctxK = {"ps": None}
                        ctxQ = {"ps": None}
                        q = interleave2(qp_ch(pp, 2, "o0"), kp_ch(pp + 1, 0, "o1"))
                        q += interleave2(qp_ch(pp, 3, "o0"), kp_ch(pp + 1, 1, "o1"))
                        q += interleave2(
                            [s for tb in (2, 3) for s in proj_chain(
                                kpt, wkt, kvTt, pp + 1, tb, rot="o0", ctx=ctxK)],
                            [s for tb in (0,) for s in proj_chain(
                                qpt, wqt, qTt, pp + 1, tb, rot="o1", ctx=ctxQ)])
                        q += qp_ch(pp + 1, 1, "o0")
                        extras[pp] = q
                        rates[(pp, 0)] = 2
                        rates[(pp, 1)] = 2
                        rates[(pp, 2)] = 3
                        rates[(pp, 3)] = 1
                    # pair 3: deferred qp in ibl0, then final W0 chains for
                    # tb0..2 in ibl1..3 (4 per psum ctx, reused sequentially)
                    q3 = [s for ch in (qp_ch(3, 2, "o0"), qp_ch(3, 3, "o1")) for s in ch]
                    for tb in range(3):
                        ctxFA, ctxFB = {"ps": None}, {"ps": None}
                        for k in range(4):
                            q3.extend(final_chain(2 * k, tb, rot="o0", ctx=ctxFA))
                            q3.extend(final_chain(2 * k + 1, tb, rot="o1", ctx=ctxFB))
                    extras[3] = q3
                    rates[(3, 0)] = 2
                    for ibl in range(1, 4):
                        rates[(3, ibl)] = 4

                    def pull(p, ibl, n):
                        q = extras.get(p, [])
                        take = q[:n]
                        extras[p] = q[n:]
                        return take

                    # ---------- attention ----------
                    for p in range(NPAIR):
                        q0 = qpt[p]
                        vslc0 = (p * VPW, p * VPW + 65)
                        vslc1 = (p * VPW + 65, (p + 1) * VPW)
                        for ibl in range(4):
                            po0 = psO.tile([65, 512], f32, tag="o0", name="po0", bufs=2)
                            po1 = psO.tile([P, 512], f32, tag="o1", name="po1", bufs=2)
                            for jg in range(NJT // 2):
                                js = (2 * jg, 2 * jg + 1)
                                ats = []
                                # up to 8 injection points per jg: 2 after
                                # each scores pair, 1 after each AV matmul
                                r = rates.get((p, ibl), 2)
                                ex = pull(p, ibl, r[jg] if isinstance(r, list) else r)
                                ei = 0
                                for j in js:
                                    sAB = psS.tile([P, 1024], f32, tag="sAB", name="sAB", bufs=2)
                                    nc.tensor.matmul(
                                        sAB[:, 0:512],
                                        lhsT=kpt[p][0:64, j * P:(j + 1) * P],
                                        rhs=q0[0:64, ibl * 512:(ibl + 1) * 512],
                                        start=True, stop=True,
                                        tile_position=(0, 0),
                                    )
                                    nc.tensor.matmul(
                                        sAB[:, 512:1024],
                                        lhsT=kpt[p][64:128, j * P:(j + 1) * P],
                                        rhs=q0[64:128, ibl * 512:(ibl + 1) * 512],
                                        start=True, stop=True,
                                        tile_position=(64, 0),
                                    )
                                    at = atp.tile([P, 1024], bf16, tag="at", name="at")
                                    ats.append(at)
                                    nc.scalar.activation(at[:, :], sAB[:, :], EXP, scale=SCALE)
                                    for _ in range(2):
                                        if ei + 4 < len(ex):
                                            ex[ei]()
                                            ei += 1
                                for j, at in zip(js, ats):
                                    nc.tensor.matmul(
                                        po0[:, :],
                                        lhsT=vp[j][:, vslc0[0]:vslc0[1]],
                                        rhs=at[:, 0:512],
                                        start=(j == 0), stop=(j == NJT - 1),
                                    )
                                    if ei < len(ex):
                                        ex[ei]()
                                        ei += 1
                                    nc.tensor.matmul(
                                        po1[:, :],
                                        lhsT=vp[j][:, vslc1[0]:vslc1[1]],
                                        rhs=at[:, 512:1024],
                                        start=(j == 0), stop=(j == NJT - 1),
                                    )
                                    if ei < len(ex):
                                        ex[ei]()
                                        ei += 1
                            # anything left in this window's queue runs now
                            for s in extras.get((p, ibl), []):
                                s()
                            extras[(p, ibl)] = []
                            # normalize: onorm[e, i] = po[e, i] / sums[i].
                            # Copy psum -> sbuf first so the po banks release
                            # early (the next ibl's AV reuses them); broadcast
                            # the reciprocals directly into both partition
                            # halves (no SBUF->SBUF shift DMA — that raced
                            # with the consumer).
                            pn0 = small.tile([65, 512], f32, tag="pn0", name="pn0")
                            pn1 = small.tile([P, 512], f32, tag="pn1", name="pn1")
                            nc.vector.tensor_copy(out=pn0[:, :], in_=po0[:, :])
                            nc.vector.tensor_copy(out=pn1[:, :], in_=po1[:, :])
                            # denominators must be staged to partition 0
                            # before the reciprocal: reciprocal_approx_fast's
                            # internal macro breaks on cross-partition in/out
                            srow0 = small.tile([1, 512], f32, tag="srow0", name="srow0")
                            nc.vector.tensor_copy(out=srow0[:, :], in_=pn0[64:65, :])
                            rrowf = small.tile([1, 1024], f32, tag="rrowf", name="rrowf")
                            nc.vector.reciprocal_approx_fast(out=rrowf[:, 0:512], in_=srow0[:, :])
                            nc.vector.reciprocal_approx_fast(out=rrowf[:, 512:1024], in_=pn1[0:1, :])
                            rbs = small.tile([P, 512], f32, tag="rbs", name="rbs")
                            rbt = small.tile([64, 512], f32, tag="rbt", name="rbt")
                            nc.gpsimd.partition_broadcast(rbs[0:64, :], rrowf[0:1, 0:512], channels=64)
                            nc.gpsimd.partition_broadcast(rbt[0:64, :], rrowf[0:1, 512:1024], channels=64)
                            nc.sync.dma_start(out=rbs[64:128, :], in_=rbt[0:64, :])
                            nc.vector.tensor_tensor(
                                out=onorm[p][0:64, ibl * 512:(ibl + 1) * 512],
                                in0=pn0[0:64, :], in1=rbs[0:64, :], op=MULT)
                            nc.vector.tensor_tensor(
                                out=onorm[p][64:128, ibl * 512:(ibl + 1) * 512],
                                in0=pn1[64:128, :], in1=rbs[64:128, :], op=MULT)
                            if dbg_d is not None and p == 0 and ibl == 0:
                                nc.sync.dma_start(out=dbg_d[0:65, 0:512], in_=pn0[:, :])
                                nc.sync.dma_start(out=dbg_d[65:193, 0:512], in_=pn1[:, :])
                                nc.sync.dma_start(out=dbg_d[193:194, 0:1024], in_=rrowf[:, :])
                                nc.sync.dma_start(
                                    out=dbg2_d[:, :],
                                    in_=onorm[0][0:128, 0:512])

                    # ---------- final W0 projection: only tb3 remains ----------
                    # 4-way interleave across the now-free psum tags so chain
                    # boundaries pipeline instead of stalling
                    tail = [final_chain(dc, 3) for dc in range(D // P)]
                    for group in (tail[0:4], tail[4:8]):
                        for i in range(NPAIR):
                            for ch in group:
                                ch[i]()

    nc.compile()
    _CACHE["nc"] = nc
    return nc


def _prep_weights(Wq, Wkv, W0):
    bf = ml_dtypes.bfloat16
    per_group = {}
    for g in range(2):
        hg = np.arange(HPC) + g * HPC            # global head ids
        d = np.arange(DH)
        # e_local = h_l*64 + d ; reference maps: e_q = d*16+h, e_k = d*32+h,
        # e_v = d*32+16+h, out channel = h*64+d
        idx_q = (d[None, :] * HEADS + hg[:, None]).reshape(-1)
        idx_k = (d[None, :] * 2 * HEADS + hg[:, None]).reshape(-1)
        idx_v = (d[None, :] * 2 * HEADS + HEADS + hg[:, None]).reshape(-1)
        idx_o = (hg[:, None] * DH + d[None, :]).reshape(-1)
        per_group[g] = {
            "wqT": np.ascontiguousarray(Wq[idx_q, :].T).astype(bf),
            "wkT": np.ascontiguousarray(Wkv[idx_k, :].T).astype(bf),
            "wvT": np.ascontiguousarray(Wkv[idx_v, :].T).astype(bf),
            "w0a": np.ascontiguousarray(W0[:, idx_o].T).astype(bf),
        }
    return per_group


def kernel(q, kv, Wq, Wkv, W0):
    from concourse.bass_utils import run_bass_kernel_spmd

    q = np.asarray(q, dtype=np.float32)
    kv = np.asarray(kv, dtype=np.float32)
    Wq = np.asarray(Wq, dtype=np.float32)
    Wkv = np.asarray(Wkv, dtype=np.float32)
    W0 = np.asarray(W0, dtype=np.float32)

    nc = _build()
    bf = ml_dtypes.bfloat16
    wg = _prep_weights(Wq, Wkv, W0)
    in_maps = []
    for c in range(8):
        b, g = divmod(c, 2)
        in_maps.append({
            "qT": np.ascontiguousarray(q[b].T).astype(bf),
            "kvT": np.ascontiguousarray(kv[b].T).astype(bf),
            "wqT": wg[g]["wqT"],
            "wkT": wg[g]["wkT"],
            "wvT": wg[g]["wvT"],
            "w0a": wg[g]["w0a"],
        })
    trace = bool(int(os.environ.get("KERNEL_TRACE", "0")))
    res = run_bass_kernel_spmd(nc, in_maps, list(range(8)), trace=trace)
    _CACHE["last_result"] = res
    sig = float(np.asarray(res.results[0]["sig"], dtype=np.float32)[0, 0])
    if abs(sig - _CACHE.get("sig_magic", 0.0)) > 0.5:
        raise RuntimeError(
            f"stale NEFF executed: sig={sig} expected {_CACHE.get('sig_magic')} "
            f"— purge /root/.neuron-compile-cache and rerun")
    out = np.empty((B, S, D), dtype=np.float32)
    for b in range(B):
        acc = (res.results[2 * b]["poutT"].astype(np.float32)
               + res.results[2 * b + 1]["poutT"].astype(np.float32))
        out[b] = acc.T
    return out


# revision 52
# speedup vs baseline: 1.0071x; 1.0071x over previous
import hashlib
import os
import numpy as np
import ml_dtypes

# nn_MultiHeadAttention: B=4, S=2048, D=1024, HEADS=16, DIM_HEAD=64.
# Sharding: batch (4) x head-group (2) across 8 cores. Each core computes
# attention for one batch and 8 heads, plus its partial of the output
# projection (row-parallel W0); the two head-group partials per batch are
# summed on the host.
#
# Scheduling notes (from trace analysis):
# - matmuls into the SAME psum bank serialize at ~373ns; alternating banks
#   pipeline at ~216ns -> all accumulation chains are 2-way interleaved.
# - the attention jg loop is ACT-bound (2x1112ns exp vs ~1500ns of matmul);
#   projection work is injected one matmul at a time between AV matmuls.
B, S, D = 4, 2048, 1024
HEADS, DH = 16, 64
HPC = 8               # heads per core
E = HPC * DH          # 512 local projection channels
SCALE = DH ** -0.5
P = 128
MT = D // P           # 8 contraction tiles
NPAIR = HPC // 2      # 4 head pairs (= e-chunks of 128)
NTB = S // 512        # 4 query blocks of 512
NJT = S // P          # 16 key tiles of 128
VPW = 65 + 128        # vp columns per pair: h0 [V|1], h1 [1|0*63|V]

_CACHE = {}


def _build():
    if "nc" in _CACHE:
        return _CACHE["nc"]
    import concourse.bacc as bacc
    import concourse.mybir as mybir
    from concourse.tile import TileContext

    f32 = mybir.dt.float32
    bf16 = mybir.dt.bfloat16
    EXP = mybir.ActivationFunctionType.Exp
    MULT = mybir.AluOpType.mult

    nc = bacc.Bacc("TRN2", target_bir_lowering=False, debug=False, num_devices=8)
    qT_d = nc.dram_tensor("qT", [D, S], bf16, kind="ExternalInput")
    kvT_d = nc.dram_tensor("kvT", [D, S], bf16, kind="ExternalInput")
    wq_d = nc.dram_tensor("wqT", [D, E], bf16, kind="ExternalInput")
    wk_d = nc.dram_tensor("wkT", [D, E], bf16, kind="ExternalInput")
    wv_d = nc.dram_tensor("wvT", [D, E], bf16, kind="ExternalInput")
    w0_d = nc.dram_tensor("w0a", [E, D], bf16, kind="ExternalInput")
    out_d = nc.dram_tensor("poutT", [D, S], bf16, kind="ExternalOutput")
    # The NEFF cache keys on the HLO signature but NOT the embedded BIR, so
    # two kernel versions with identical I/O signatures collide and a stale
    # NEFF gets silently reused. Encode a hash of this source file into a
    # dummy output's shape so every kernel edit changes the signature.
    try:
        with open(__file__, "rb") as f:
            _h = int(hashlib.sha256(f.read()).hexdigest()[:8], 16)
    except OSError:
        _h = 0
    SIG_MAGIC = float(1000 + _h % 509)
    _CACHE["sig_magic"] = SIG_MAGIC
    sig_d = nc.dram_tensor("sig", [1, 2 + _h % 509], f32, kind="ExternalOutput")
    DEBUG_DUMP = bool(int(os.environ.get("KERNEL_DEBUG_DUMP", "0")))
    dbg_d = dbg2_d = None
    if DEBUG_DUMP:
        # rows 0:65 pn0, 65:193 pn1, 193 rrowf(1024)
        dbg_d = nc.dram_tensor("dbg", [323, 1024], f32, kind="ExternalOutput")
        dbg2_d = nc.dram_tensor("dbg2", [P, 512], bf16, kind="ExternalOutput")

    with TileContext(nc) as tc:
        with (
            tc.tile_pool(name="pers", bufs=1) as pers,
            tc.tile_pool(name="psS", bufs=1, space="PSUM") as psS,
            tc.tile_pool(name="psO", bufs=1, space="PSUM") as psO,
        ):
            # ---- persistent SBUF tiles (live across phases) ----
            sig_t = pers.tile([1, 8], f32, tag="sig", name="sig")
            nc.gpsimd.memset(sig_t[:, :], SIG_MAGIC)
            nc.sync.dma_start(out=sig_d[0:1, 0:2], in_=sig_t[0:1, 0:2])
            w0a = [pers.tile([P, D], bf16, tag=f"w0{p}", name=f"w0{p}") for p in range(NPAIR)]
            qpt = [pers.tile([P, S], bf16, tag=f"qp{p}", name=f"qp{p}") for p in range(NPAIR)]
            kpt = [pers.tile([P, S], bf16, tag=f"kp{p}", name=f"kp{p}") for p in range(NPAIR)]
            vp = [pers.tile([P, NPAIR * VPW], bf16, tag=f"vp{t}", name=f"vp{t}") for t in range(NJT)]
            onorm = [pers.tile([P, S], bf16, tag=f"on{p}", name=f"on{p}") for p in range(NPAIR)]
            for p in range(NPAIR):
                nc.sync.dma_start(out=w0a[p][:, :], in_=w0_d[p * P:(p + 1) * P, :])

            # psum allocation for projection chains. Outside attention all
            # tags are free (5-entry rotation); during attention a chain must
            # use an explicit po tag (o0/o1) — its other buffer — since a
            # long-lived chain on a sAB buffer would stall the scores/exp
            # pipeline. At most one chain alloc per tag per ibl window, or the
            # alloc rotation hands out the live po buffer and deadlocks.
            ptags_free = [(psO, "o0"), (psS, "sAB"), (psO, "o1"),
                          (psS, "sAB"), (psS, "sAB")]
            pi_state = [0]

            def proj_psum(sel="free"):
                if sel == "free":
                    pool, tag = ptags_free[pi_state[0] % len(ptags_free)]
                    pi_state[0] += 1
                else:
                    pool, tag = psO, sel
                return pool.tile([P, 512], f32, tag=tag, name=f"proj_{tag}", bufs=2)

            with tc.tile_pool(name="phA", bufs=1) as pha:
                qTt = [pha.tile([P, S], bf16, tag=f"qT{i}", name=f"qT{i}") for i in range(MT)]
                kvTt = [pha.tile([P, S], bf16, tag=f"kvT{i}", name=f"kvT{i}") for i in range(MT)]
                wqt = [pha.tile([P, E], bf16, tag=f"wq{i}", name=f"wq{i}") for i in range(MT)]
                wkt = [pha.tile([P, E], bf16, tag=f"wk{i}", name=f"wk{i}") for i in range(MT)]
                wvt = [pha.tile([P, E], bf16, tag=f"wv{i}", name=f"wv{i}") for i in range(MT)]

                # DMA in consumption order: wv + kvT column-block 0 feed the
                # first vp chains; wk then the kp chains; then the rest.
                # alternate input loads across both HWDGE queues (sync +
                # scalar engines) — the pre-phase is input-bandwidth-bound
                # and the scalar engine is idle until the first exp
                def in_dma(i, out, src):
                    eng = nc.sync if i % 2 == 0 else nc.scalar
                    eng.dma_start(out=out, in_=src)

                for i in range(MT):
                    in_dma(i, wvt[i][:, :], wv_d[i * P:(i + 1) * P, :])
                for i in range(MT):
                    in_dma(i, kvTt[i][:, 0:512], kvT_d[i * P:(i + 1) * P, 0:512])
                for i in range(MT):
                    in_dma(i, wkt[i][:, :], wk_d[i * P:(i + 1) * P, :])
                for i in range(MT):
                    in_dma(i, kvTt[i][:, 512:1024], kvT_d[i * P:(i + 1) * P, 512:1024])
                for i in range(MT):
                    in_dma(i, kvTt[i][:, 1024:2048], kvT_d[i * P:(i + 1) * P, 1024:2048])
                for i in range(MT):
                    in_dma(i, wqt[i][:, :], wq_d[i * P:(i + 1) * P, :])
                    in_dma(i + 1, qTt[i][:, 0:512], qT_d[i * P:(i + 1) * P, 0:512])
                # late loads overlap attention: sync queue only, so the
                # scalar engine's DMA slices don't steal exp time
                for i in range(MT):
                    nc.sync.dma_start(out=qTt[i][:, 512:S], in_=qT_d[i * P:(i + 1) * P, 512:S])

                # ---------- chain generators: yield one step per call ----------
                # A chain = psum alloc + 8 (or 4) accumulating matmuls + finish.
                # Steps from two live chains are interleaved so consecutive
                # matmuls hit different psum banks.

                def vp_chain(t, rot="free", ctx=None):
                    st = ctx if ctx is not None else {"ps": None}

                    def step(i):
                        if i == 0:
                            nc.gpsimd.memset(vp[t][:, :], 0.0)
                            v3 = vp[t].rearrange("x (g c) -> x g c", c=VPW)
                            nc.gpsimd.memset(v3[:, :, 64:66], 1.0)
                            if st["ps"] is None:
                                st["ps"] = proj_psum(rot)
                        nc.tensor.matmul(
                            st["ps"][:, :],
                            lhsT=kvTt[i][:, t * P:(t + 1) * P],
                            rhs=wvt[i][:, :],
                            start=(i == 0), stop=(i == MT - 1),
                        )
                        if i == MT - 1:
                            v3 = vp[t].rearrange("x (g c) -> x g c", c=VPW)
                            p3 = st["ps"].rearrange("x (g c) -> x g c", c=P)
                            nc.vector.tensor_copy(out=v3[:, :, 0:64], in_=p3[:, :, 0:64])
                            nc.vector.tensor_copy(out=v3[:, :, 129:193], in_=p3[:, :, 64:128])
                    return [lambda i=i: step(i) for i in range(MT)]

                def proj_chain(dst, wt, xt, ec, tb, rot="free", ctx=None):
                    st = ctx if ctx is not None else {"ps": None}

                    def step(i):
                        if i == 0 and st["ps"] is None:
                            st["ps"] = proj_psum(rot)
                        nc.tensor.matmul(
                            st["ps"][:, :],
                            lhsT=wt[i][:, ec * P:(ec + 1) * P],
                            rhs=xt[i][:, tb * 512:(tb + 1) * 512],
                            start=(i == 0), stop=(i == MT - 1),
                        )
                        if i == MT - 1:
                            nc.vector.tensor_copy(
                                out=dst[ec][:, tb * 512:(tb + 1) * 512], in_=st["ps"][:, :])
                    return [lambda i=i: step(i) for i in range(MT)]

                def final_chain(dc, tb, rot="free", ctx=None):
                    # ctx: a {"ps": tile-or-None} holder shared by several
                    # sequential final chains; reusing one psum alloc avoids
                    # extra pool-rotation allocs (which can deadlock against a
                    # live po buffer mid-ibl).
                    st = ctx if ctx is not None else {"ps": None}

                    def step(i):
                        if i == 0 and st["ps"] is None:
                            st["ps"] = proj_psum(rot)
                        nc.tensor.matmul(
                            st["ps"][:, :],
                            lhsT=w0a[i][:, dc * P:(dc + 1) * P],
                            rhs=onorm[i][:, tb * 512:(tb + 1) * 512],
                            start=(i == 0), stop=(i == NPAIR - 1),
                        )
                        if i == NPAIR - 1:
                            ob = obp.tile([P, 512], bf16, tag="ob", name="ob")
                            nc.vector.tensor_copy(out=ob[:, :], in_=st["ps"][:, :])
                            nc.sync.dma_start(
                                out=out_d[dc * P:(dc + 1) * P, tb * 512:(tb + 1) * 512],
                                in_=ob[:, :])
                    return [lambda i=i: step(i) for i in range(NPAIR)]

                def run_pair(chA, chB, interleave=True):
                    if not interleave:
                        for s in chA:
                            s()
                        for s in chB:
                            s()
                        return
                    la, lb = len(chA), len(chB)
                    for i in range(max(la, lb)):
                        if i < la:
                            chA[i]()
                        if i < lb:
                            chB[i]()

                with (
                    tc.tile_pool(name="at", bufs=3) as atp,
                    tc.tile_pool(name="small", bufs=2) as small,
                    tc.tile_pool(name="ob", bufs=3) as obp,
                ):
                    # ---------- pre-phase: the minimum pair-0 ibl-0 needs ----------
                    # vp8..13 are deferred into ibl0's slack (each completes a
                    # jg before its AV consumer); vp14/15 stay here since the
                    # injection rate can't finish them by jg6.
                    ILV_PRE = True
                    for t in range(0, 8, 2):
                        run_pair(vp_chain(t), vp_chain(t + 1), interleave=ILV_PRE)
                    run_pair(proj_chain(kpt, wkt, kvTt, 0, 0),
                             proj_chain(kpt, wkt, kvTt, 0, 1), interleave=ILV_PRE)
                    run_pair(proj_chain(kpt, wkt, kvTt, 0, 2),
                             proj_chain(kpt, wkt, kvTt, 0, 3), interleave=ILV_PRE)
                    run_pair(vp_chain(14), vp_chain(15), interleave=ILV_PRE)
                    run_pair(proj_chain(qpt, wqt, qTt, 0, 0),
                             proj_chain(qpt, wqt, qTt, 0, 1), interleave=ILV_PRE)

                    # ---------- extras: per-(pair, ibl) step queues ----------
                    # During pair p's attention we inject projection work one
                    # matmul at a time between AV matmuls. Each ibl window has
                    # two lanes (psum tags o0/o1, the po buffers' second
                    # buffer); a lane's chains run sequentially, lanes are
                    # interleaved element-wise for psum-bank alternation.
                    def kp_ch(pn, tb, tag):
                        return proj_chain(kpt, wkt, kvTt, pn, tb, rot=tag)

                    def qp_ch(pn, tb, tag):
                        return proj_chain(qpt, wqt, qTt, pn, tb, rot=tag)

                    # flat queue per pair; chains run sequentially (one live
                    # at a time), alternating psum tags o0/o1 per chain. Pull
                    # counts per ibl are sized so each chain lands in the ibl
                    # window its outputs are needed after (and finals only
                    # start once their onorm block exists).
                    def interleave2(qA, qB):
                        q = []
                        for i in range(max(len(qA), len(qB))):
                            if i < len(qA):
                                q.append(qA[i])
                            if i < len(qB):
                                q.append(qB[i])
                        return q

                    extras = {}
                    rates = {}
                    # pair 0 ibl0: vp8..13 on two reused psum ctxs at 8/jg
                    ctxA, ctxB = {"ps": None}, {"ps": None}
                    qA, qB = [], []
                    for t in range(8, 14, 2):
                        qA.extend(vp_chain(t, rot="o0", ctx=ctxA))
                        qB.extend(vp_chain(t + 1, rot="o1", ctx=ctxB))
                    q0 = interleave2(qA, qB)
                    # ibl1: kp1 tb0+tb1 share one o0 ctx; qp0-tb2 on o1.
                    # ibl2 likewise with kp1 tb2+tb3 and qp0-tb3.
                    ctxK1, ctxK2 = {"ps": None}, {"ps": None}
                    q0 += interleave2(
                        [s for tb in (0, 1) for s in proj_chain(
                            kpt, wkt, kvTt, 1, tb, rot="o0", ctx=ctxK1)],
                        qp_ch(0, 2, "o1"))
                    ctxQ0 = {"ps": None}
                    q0 += interleave2(
                        [s for tb in (2, 3) for s in proj_chain(
                            kpt, wkt, kvTt, 1, tb, rot="o0", ctx=ctxK2)],
                        [s for args in ((0, 3), (1, 0)) for s in proj_chain(
                            qpt, wqt, qTt, args[0], args[1], rot="o1", ctx=ctxQ0)])
                    q0 += qp_ch(1, 1, "o0")
                    extras[0] = q0
                    rates[(0, 0)] = [8, 8, 8, 8, 8, 8, 0, 0]
                    rates[(0, 1)] = 3
                    rates[(0, 2)] = 4
                    rates[(0, 3)] = 1
                    for pp in (1, 2):
                        ctxK = {"ps": None}
                        ctxQ = {"ps": None}
                        q = interleave2(qp_ch(pp, 2, "o0"), kp_ch(pp + 1, 0, "o1"))
                        q += interleave2(qp_ch(pp, 3, "o0"), kp_ch(pp + 1, 1, "o1"))
                        q += interleave2(
                            [s for tb in (2, 3) for s in proj_chain(
                                kpt, wkt, kvTt, pp + 1, tb, rot="o0", ctx=ctxK)],
                            [s for tb in (0,) for s in proj_chain(
                                qpt, wqt, qTt, pp + 1, tb, rot="o1", ctx=ctxQ)])
                        q += qp_ch(pp + 1, 1, "o0")
                        extras[pp] = q
                        rates[(pp, 0)] = 2
                        rates[(pp, 1)] = 2
                        rates[(pp, 2)] = 3
                        rates[(pp, 3)] = 1
                    # pair 3: deferred qp in ibl0, then final W0 chains for
                    # tb0..2 in ibl1..3 (4 per psum ctx, reused sequentially)
                    q3 = [s for ch in (qp_ch(3, 2, "o0"), qp_ch(3, 3, "o1")) for s in ch]
                    for tb in range(3):
                        ctxFA, ctxFB = {"ps": None}, {"ps": None}
                        for k in range(4):
                            q3.extend(final_chain(2 * k, tb, rot="o0", ctx=ctxFA))
                            q3.extend(final_chain(2 * k + 1, tb, rot="o1", ctx=ctxFB))
                    extras[3] = q3
                    rates[(3, 0)] = 2
                    for ibl in range(1, 4):
                        rates[(3, ibl)] = 4

                    def pull(p, ibl, n):
                        q = extras.get(p, [])
                        take = q[:n]
                        extras[p] = q[n:]
                        return take

                    # ---------- attention ----------
                    for p in range(NPAIR):
                        q0 = qpt[p]
                        vslc0 = (p * VPW, p * VPW + 65)
                        vslc1 = (p * VPW + 65, (p + 1) * VPW)
                        for ibl in range(4):
                            po0 = psO.tile([65, 512], f32, tag="o0", name="po0", bufs=2)
                            po1 = psO.tile([P, 512], f32, tag="o1", name="po1", bufs=2)
                            for jg in range(NJT // 2):
                                js = (2 * jg, 2 * jg + 1)
                                ats = []
                                # up to 8 injection points per jg: 2 after
                                # each scores pair, 1 after each AV matmul
                                r = rates.get((p, ibl), 2)
                                ex = pull(p, ibl, r[jg] if isinstance(r, list) else r)
                                ei = 0
                                for j in js:
                                    sAB = psS.tile([P, 1024], f32, tag="sAB", name="sAB", bufs=2)
                                    nc.tensor.matmul(
                                        sAB[:, 0:512],
                                        lhsT=kpt[p][0:64, j * P:(j + 1) * P],
                                        rhs=q0[0:64, ibl * 512:(ibl + 1) * 512],
                                        start=True, stop=True,
                                        tile_position=(0, 0),
                                    )
                                    nc.tensor.matmul(
                                        sAB[:, 512:1024],
                                        lhsT=kpt[p][64:128, j * P:(j + 1) * P],
                                        rhs=q0[64:128, ibl * 512:(ibl + 1) * 512],
                                        start=True, stop=True,
                                        tile_position=(64, 0),
                                    )
                                    at = atp.tile([P, 1024], bf16, tag="at", name="at")
                                    ats.append(at)
                                    nc.scalar.activation(at[:, :], sAB[:, :], EXP, scale=SCALE)
                                    for _ in range(2):
                                        if ei + 4 < len(ex):
                                            ex[ei]()
                                            ei += 1
                                for j, at in zip(js, ats):
                                    nc.tensor.matmul(
                                        po0[:, :],
                                        lhsT=vp[j][:, vslc0[0]:vslc0[1]],
                                        rhs=at[:, 0:512],
                                        start=(j == 0), stop=(j == NJT - 1),
                                    )
                                    if ei < len(ex):
                                        ex[ei]()
                                        ei += 1
                                    nc.tensor.matmul(
                                        po1[:, :],
                                        lhsT=vp[j][:, vslc1[0]:vslc1[1]],
                                        rhs=at[:, 512:1024],
                                        start=(j == 0), stop=(j == NJT - 1),
                                    )
                                    if ei < len(ex):
                                        ex[ei]()
                                        ei += 1
                            # anything left in this window's queue runs now
                            for s in extras.get((p, ibl), []):
                                s()
                            extras[(p, ibl)] = []
                            # normalize: onorm[e, i] = po[e, i] / sums[i].
                            # Copy psum -> sbuf first so the po banks release
                            # early (the next ibl's AV reuses them); broadcast
                            # the reciprocals directly into both partition
                            # halves (no SBUF->SBUF shift DMA — that raced
                            # with the consumer).
                            pn0 = small.tile([65, 512], f32, tag="pn0", name="pn0")
                            pn1 = small.tile([P, 512], f32, tag="pn1", name="pn1")
                            nc.vector.tensor_copy(out=pn0[:, :], in_=po0[:, :])
                            nc.vector.tensor_copy(out=pn1[:, :], in_=po1[:, :])
                            # denominators must be staged to partition 0
                            # before the reciprocal: reciprocal_approx_fast's
                            # internal macro breaks on cross-partition in/out
                            srow0 = small.tile([1, 512], f32, tag="srow0", name="srow0")
                            nc.vector.tensor_copy(out=srow0[:, :], in_=pn0[64:65, :])
                            rrowf = small.tile([1, 1024], f32, tag="rrowf", name="rrowf")
                            nc.vector.reciprocal_approx_fast(out=rrowf[:, 0:512], in_=srow0[:, :])
                            nc.vector.reciprocal_approx_fast(out=rrowf[:, 512:1024], in_=pn1[0:1, :])
                            rbs = small.tile([P, 512], f32, tag="rbs", name="rbs")
                            rbt = small.tile([64, 512], f32, tag="rbt", name="rbt")
                            nc.gpsimd.partition_broadcast(rbs[0:64, :], rrowf[0:1, 0:512], channels=64)
                            nc.gpsimd.partition_broadcast(rbt[0:64, :], rrowf[0:1, 512:1024], channels=64)
                            nc.sync.dma_start(out=rbs[64:128, :], in_=rbt[0:64, :])
                            nc.vector.tensor_tensor(
                                out=onorm[p][0:64, ibl * 512:(ibl + 1) * 512],
                                in0=pn0[0:64, :], in1=rbs[0:64, :], op=MULT)
                            nc.vector.tensor_tensor(
                                out=onorm[p][64:128, ibl * 512:(ibl + 1) * 512],
                                in0=pn1[64:128, :], in1=rbs[64:128, :], op=MULT)
                            if dbg_d is not None and p == 0 and ibl == 0:
                                nc.sync.dma_start(out=dbg_d[0:65, 0:512], in_=pn0[:, :])
                                nc.sync.dma_start(out=dbg_d[65:193, 0:512], in_=pn1[:, :])
                                nc.sync.dma_start(out=dbg_d[193:194, 0:1024], in_=rrowf[:, :])
                                nc.sync.dma_start(
                                    out=dbg2_d[:, :],
                                    in_=onorm[0][0:128, 0:512])

                    # ---------- final W0 projection: only tb3 remains ----------
                    for dc in range(0, D // P, 2):
                        run_pair(final_chain(dc, 3), final_chain(dc + 1, 3),
                                 interleave=ILV_PRE)

    nc.compile()
    _CACHE["nc"] = nc
    return nc


def _prep_weights(Wq, Wkv, W0):
    bf = ml_dtypes.bfloat16
    per_group = {}
    for g in range(2):
        hg = np.arange(HPC) + g * HPC            # global head ids
        d = np.arange(DH)
        # e_local = h_l*64 + d ; reference maps: e_q = d*16+h, e_k = d*32+h,
        # e_v = d*32+16+h, out channel = h*64+d
        idx_q = (d[None, :] * HEADS + hg[:, None]).reshape(-1)
        idx_k = (d[None, :] * 2 * HEADS + hg[:, None]).reshape(-1)
        idx_v = (d[None, :] * 2 * HEADS + HEADS + hg[:, None]).reshape(-1)
        idx_o = (hg[:, None] * DH + d[None, :]).reshape(-1)
        per_group[g] = {
            "wqT": np.ascontiguousarray(Wq[idx_q, :].T).astype(bf),
            "wkT": np.ascontiguousarray(Wkv[idx_k, :].T).astype(bf),
            "wvT": np.ascontiguousarray(Wkv[idx_v, :].T).astype(bf),
            "w0a": np.ascontiguousarray(W0[:, idx_o].T).astype(bf),
        }
    return per_group


def kernel(q, kv, Wq, Wkv, W0):
    from concourse.bass_utils import run_bass_kernel_spmd

    q = np.asarray(q, dtype=np.float32)
    kv = np.asarray(kv, dtype=np.float32)
    Wq = np.asarray(Wq, dtype=np.float32)
    Wkv = np.asarray(Wkv, dtype=np.float32)
    W0 = np.asarray(W0, dtype=np.float32)

    nc = _build()
    bf = ml_dtypes.bfloat16
    wg = _prep_weights(Wq, Wkv, W0)
    in_maps = []
    for c in range(8):
        b, g = divmod(c, 2)
        in_maps.append({
            "qT": np.ascontiguousarray(q[b].T).astype(bf),
            "kvT": np.ascontiguousarray(kv[b].T).astype(bf),
            "wqT": wg[g]["wqT"],
            "wkT": wg[g]["wkT"],
            "wvT": wg[g]["wvT"],
            "w0a": wg[g]["w0a"],
        })
    trace = bool(int(os.environ.get("KERNEL_TRACE", "0")))
    res = run_bass_kernel_spmd(nc, in_maps, list(range(8)), trace=trace)
    _CACHE["last_result"] = res
    sig = float(np.asarray(res.results[0]["sig"], dtype=np.float32)[0, 0])
    if abs(sig - _CACHE.get("sig_magic", 0.0)) > 0.5:
        raise RuntimeError(
            f"stale NEFF executed: sig={sig} expected {_CACHE.get('sig_magic')} "
            f"— purge /root/.neuron-compile-cache and rerun")
    out = np.empty((B, S, D), dtype=np.float32)
    for b in range(B):
        acc = (res.results[2 * b]["poutT"].astype(np.float32)
               + res.results[2 * b + 1]["poutT"].astype(np.float32))
        out[b] = acc.T
    return out
